# revision 7
# baseline (speedup 1.0000x reference)
"""GCN encoder kernel for 8 Trainium2 NeuronCores.

Strategy
--------
out = relu(relu(A_hat @ x @ W0) @ W1), A_hat = D^-1/2 (A + I) D^-1/2.

- Destinations (output rows) are sharded across the 8 cores; each core owns
  N/8 nodes and all edges pointing at them.
- Host-side prep (index work only): per core, edges are bucketed by
  destination, destinations are degree-sorted into tiles of 128, and each
  edge becomes a "slot" (partition = destination's position in its tile,
  column = edge rank).  Slots are gathered from HBM with dma_gather using
  node-PAIR rows (512 B) so the int16 index (= src//2) covers all 50k nodes;
  a per-slot norm pair masks the wanted half.  Per-edge norm
  dinv[src]*dinv[dst] rides in that mask, so the device computes the full
  normalized aggregation.  Self-loop terms skip the gather: the core's own
  x rows arrive position-ordered and are scaled by dinv^2 on device.
- On device: dma_gather over 4 SWDGE queues (the gather is the bottleneck;
  multiple queues overlap ring drain), DVE applies the norm mask and folds
  the pair halves, TensorE accumulates slot columns into PSUM quarters via
  an identity stationary (segment-sum), then the two dense layers run
  feature-major with fused ReLU eviction on ScalarE.
"""

import os
import sys

for _p in ("/opt/trn_rl_repo", "/root/.axon_site/_ro/trn_rl_repo"):
    if os.path.isdir(_p) and _p not in sys.path:
        sys.path.insert(0, _p)

import numpy as np
import ml_dtypes
from contextlib import ExitStack

import concourse.bass as bass
import concourse.tile as tile
from concourse import bacc, mybir
from concourse.bass_utils import run_bass_kernel_spmd
from concourse.masks import make_identity
from concourse.ap import AP

P = 128
NCORES = 8
CALL_COLS = 12         # max slot-columns per dma_gather call (1536 slots)
NQ = 4                 # SWDGE queues
bf16 = mybir.dt.bfloat16
f32 = mybir.dt.float32
i16 = mybir.dt.int16
BF = ml_dtypes.bfloat16


def _ap3(t_ap, d1, d2):
    st = t_ap.ap[-1][0]
    return AP(t_ap.tensor, t_ap.offset, [t_ap.ap[0], [d2 * st, d1], [st, d2]])


def _prep(x, W0, W1, edge_index):
    N, F = x.shape
    H = W0.shape[1]
    ND = (N + NCORES - 1) // NCORES          # dsts per core
    NT = (ND + P - 1) // P                   # dst tiles per core
    NDP = NT * P                             # padded dsts per core

    row = np.asarray(edge_index[0], dtype=np.int64)
    col = np.asarray(edge_index[1], dtype=np.int64)
    deg = np.bincount(col, minlength=N).astype(np.float32) + 1.0
    dinv = (1.0 / np.sqrt(deg)).astype(np.float32)

    norm_e = dinv[row] * dinv[col]
    core_of = col // ND

    npair = (N + 1) // 2 + 1                 # +1 zero pair
    zero_pair = npair - 1
    assert zero_pair <= 32767

    xp = np.zeros((2 * npair, F), dtype=BF)
    xp[:N] = x.astype(BF)
    ypair = xp.reshape(npair, 2 * F)

    per_core = []
    sdeg_tiles = np.zeros((NCORES, NT), dtype=np.int64)
    for c in range(NCORES):
        m = core_of == c
        r = row[m]
        dl = col[m] - c * ND
        nm = norm_e[m]
        key = dl * npair + (r >> 1)
        uniq, inv = np.unique(key, return_inverse=True)
        S0 = uniq.shape[0]
        norm2 = np.zeros((S0, 2), dtype=np.float32)
        np.add.at(norm2, (inv, (r & 1).astype(np.int64)), nm)
        slot_dl = (uniq // npair).astype(np.int64)
        slot_pr = (uniq % npair).astype(np.int64)
        sdeg = np.bincount(slot_dl, minlength=NDP)
        start_of = np.zeros(NDP + 1, dtype=np.int64)
        np.cumsum(sdeg, out=start_of[1:])
        j_rank = np.arange(S0, dtype=np.int64) - start_of[slot_dl]
        perm = np.argsort(-sdeg, kind="stable")       # position -> dst
        pos_of = np.empty(NDP, dtype=np.int64)
        pos_of[perm] = np.arange(NDP)
        sdeg_tiles[c] = sdeg[perm].reshape(NT, P).max(axis=1)
        per_core.append(dict(slot_dl=slot_dl, slot_pr=slot_pr, j_rank=j_rank,
                             norm2=norm2, pos_of=pos_of, perm=perm))

    cols_t = sdeg_tiles.max(axis=0).astype(np.int64)  # ragged, may be 0
    colbase = np.zeros(NT + 1, dtype=np.int64)
    np.cumsum(cols_t, out=colbase[1:])
    C = int(colbase[-1])

    # per-tile gather calls of <= CALL_COLS columns
    calls = []   # (tile, col_lo_global, ncols)
    for t in range(NT):
        c0 = int(colbase[t])
        left = int(cols_t[t])
        while left > 0:
            w = min(CALL_COLS, left)
            calls.append((t, c0, w))
            c0 += w
            left -= w

    in_maps = []
    unshard = []
    for c in range(NCORES):
        pc = per_core[c]
        pos = pc["pos_of"][pc["slot_dl"]]
        prow = pos % P
        scol = colbase[pos // P] + pc["j_rank"]
        idx_arr = np.full((P, max(C, 1)), zero_pair, dtype=np.int16)
        idx_arr[prow, scol] = pc["slot_pr"].astype(np.int16)
        norm2_arr = np.zeros((P, 2 * max(C, 1)), dtype=BF)
        norm2_arr[prow, 2 * scol] = pc["norm2"][:, 0].astype(BF)
        norm2_arr[prow, 2 * scol + 1] = pc["norm2"][:, 1].astype(BF)
        # idx re-layout: per call, slot i (= colj*128 + p over the call's
        # columns) lives at [i%16, base*8 + i//16], replicated over 8 row-groups
        blocks = []
        for (_t, c0, w) in calls:
            blk = idx_arr[:, c0:c0 + w]                       # [128, w]
            v = blk.T.reshape(-1)                             # slot-major
            b = v.reshape(w * 8, 16).T                        # [16, w*8]
            blocks.append(np.tile(b, (8, 1)))
        idx16 = np.concatenate(blocks, axis=1) if blocks else np.zeros((P, 8), np.int16)
        # self-loop inputs: x rows in position order + dinv^2 per position
        nd_c = min(ND, N - c * ND)
        xs = np.zeros((NDP, F), dtype=BF)
        d2 = np.zeros(NDP, dtype=np.float32)
        valid = pc["perm"] < nd_c
        gids = c * ND + pc["perm"][valid]
        xs[valid] = x[gids].astype(BF)
        d2[valid] = dinv[gids] ** 2
        xself = np.ascontiguousarray(
            xs.reshape(NT, P, F).transpose(1, 0, 2).reshape(P, NT * F))
        dinv2 = np.ascontiguousarray(
            d2.reshape(NT, P).T.astype(BF))                   # [128, NT]
        in_maps.append({
            "ypair": ypair,
            "idx": np.ascontiguousarray(idx16),
            "norm2": np.ascontiguousarray(norm2_arr),
            "xself": xself,
            "dinv2": dinv2,
            "w0": W0.astype(BF),
            "w1lo": W1[:128].astype(BF),
            "w1hi": W1[128:].astype(BF),
        })
        unshard.append(pc["pos_of"])

    meta = dict(N=N, F=F, H=H, ND=ND, NT=NT, NDP=NDP, npair=npair,
                C=max(C, 1), cols_t=cols_t.tolist(), colbase=colbase.tolist(),
                calls=calls, idx_cols=sum(w * 8 for (_t, _c, w) in calls))
    return in_maps, unshard, meta


def _build(meta):
    F, H = meta["F"], meta["H"]
    NT, npair = meta["NT"], meta["npair"]
    C, cols_t, colbase = meta["C"], meta["cols_t"], meta["colbase"]
    calls = meta["calls"]
    idx_cols = meta["idx_cols"]
    F2 = 2 * F

    nc = bacc.Bacc(None, target_bir_lowering=False, debug=False,
                   num_devices=NCORES, num_swdge_queues=NQ,
                   dynamic_dma_scratch_size=NQ * CALL_COLS * P * 16)
    ypair_d = nc.declare_dram_parameter("ypair", [npair, F2], bf16, isOutput=False)
    idx_d = nc.declare_dram_parameter("idx", [P, idx_cols], i16, isOutput=False)
    norm2_d = nc.declare_dram_parameter("norm2", [P, 2 * C], bf16, isOutput=False)
    xself_d = nc.declare_dram_parameter("xself", [P, NT * F], bf16, isOutput=False)
    dinv2_d = nc.declare_dram_parameter("dinv2", [P, NT], bf16, isOutput=False)
    w0_d = nc.declare_dram_parameter("w0", [F, H], bf16, isOutput=False)
    w1lo_d = nc.declare_dram_parameter("w1lo", [128, H], bf16, isOutput=False)
    w1hi_d = nc.declare_dram_parameter("w1hi", [H - 128, H], bf16, isOutput=False)
    out_d = nc.declare_dram_parameter("out", [H, NT * P], f32, isOutput=True)

    chunks = [(j * 4, min(4, NT - j * 4)) for j in range((NT + 3) // 4)]

    with tile.TileContext(nc) as tc, ExitStack() as ctx:
        cpool = ctx.enter_context(tc.tile_pool(name="const", bufs=1))
        gpool = ctx.enter_context(tc.tile_pool(name="g", bufs=2))
        spool = ctx.enter_context(tc.tile_pool(name="gs", bufs=2))
        s2pool = ctx.enter_context(tc.tile_pool(name="gs2", bufs=2))
        hpool = ctx.enter_context(tc.tile_pool(name="h0", bufs=2))
        h0Tp = ctx.enter_context(tc.tile_pool(name="h0T", bufs=3))
        h1p = ctx.enter_context(tc.tile_pool(name="h1", bufs=2))
        opool = ctx.enter_context(tc.tile_pool(name="o", bufs=2))
        ps_acc = ctx.enter_context(tc.tile_pool(name="ps_acc", bufs=2, space="PSUM"))
        ps_tr = ctx.enter_context(tc.tile_pool(name="ps_tr", bufs=2, space="PSUM"))
        ps_u = ctx.enter_context(tc.tile_pool(name="ps_u", bufs=1, space="PSUM"))
        ps_v = ctx.enter_context(tc.tile_pool(name="ps_v", bufs=1, space="PSUM"))

        ident = cpool.tile([P, P], bf16)
        make_identity(nc, ident[:])
        # split the index/norm prologue loads so the first gathers start early
        idx_sb = cpool.tile([P, idx_cols], i16)
        n_head = min(idx_cols, 16 * 8 * 4)
        nc.sync.dma_start(idx_sb[:, :n_head], idx_d[:, :n_head])
        if idx_cols > n_head:
            nc.sync.dma_start(idx_sb[:, n_head:], idx_d[:, n_head:])
        norm2_sb = cpool.tile([P, 2 * C], bf16)
        c_head = min(2 * C, 2 * 64)
        nc.sync.dma_start(norm2_sb[:, :c_head], norm2_d[:, :c_head])
        if 2 * C > c_head:
            nc.sync.dma_start(norm2_sb[:, c_head:], norm2_d[:, c_head:])
        xself_sb = cpool.tile([P, NT * F], bf16)
        nc.sync.dma_start(xself_sb[:], xself_d[:])
        dinv2_sb = cpool.tile([P, NT], bf16)
        nc.sync.dma_start(dinv2_sb[:], dinv2_d[:])
        w0_sb = cpool.tile([F, H], bf16)
        nc.sync.dma_start(w0_sb[:], w0_d[:])
        w1lo_sb = cpool.tile([128, H], bf16)
        nc.sync.dma_start(w1lo_sb[:], w1lo_d[:])
        w1hi_sb = cpool.tile([H - 128, H], bf16)
        nc.sync.dma_start(w1hi_sb[:], w1hi_d[:])

        # self-loop columns: xself * dinv2 (broadcast along features), in place
        selfcols = xself_sb
        nc.vector.tensor_tensor(out=selfcols[:], in0=xself_sb[:],
                                in1=dinv2_sb[:].to_broadcast([P, NT, F]),
                                op=mybir.AluOpType.mult)

        h0T_chunk = {}

        def finish_tile(t, accp, nquad):
            h0tmp = hpool.tile([P, P], bf16, tag="h0tmp")
            in_ap = AP(accp[:].tensor, accp[:].offset,
                       [accp[:].ap[0], [1, P], [P, nquad]])
            with nc.allow_low_precision("bf16 h0 evict"):
                nc.vector.tensor_reduce(h0tmp[:], in_ap, axis=mybir.AxisListType.X,
                                        op=mybir.AluOpType.add, opt_input=False)
            trp = ps_tr.tile([P, P], bf16, tag="tr")
            nc.tensor.transpose(trp[:], h0tmp[:], ident[:])
            j = t // 4
            if j not in h0T_chunk:
                w = chunks[j][1] * P
                h0T_new = h0Tp.tile([P, w], bf16, tag="h0T")
                h0T_chunk[j] = h0T_new
            nc.scalar.copy(h0T_chunk[j][:, (t % 4) * P:(t % 4 + 1) * P], trp[:])
            if t % 4 == 3 or t == NT - 1:
                phase2(j)

        def phase2(j):
            t0, ntile = chunks[j]
            w = ntile * P
            h0T = h0T_chunk.pop(j)
            u1 = ps_u.tile([P, w], f32, tag="u1")
            u2 = ps_u.tile([P, w], f32, tag="u2")
            nc.tensor.matmul(u1[:], lhsT=w0_sb[:, 0:128], rhs=h0T[:], start=True, stop=True)
            nc.tensor.matmul(u2[:], lhsT=w0_sb[:, 128:H], rhs=h0T[:], start=True, stop=True)
            h1a = h1p.tile([P, w], bf16, tag="h1a")
            h1b = h1p.tile([P, w], bf16, tag="h1b")
            nc.scalar.activation(h1a[:], u1[:], mybir.ActivationFunctionType.Relu)
            nc.scalar.activation(h1b[:], u2[:], mybir.ActivationFunctionType.Relu)
            v1 = ps_v.tile([P, w], f32, tag="v1")
            v2 = ps_v.tile([P, w], f32, tag="v2")
            nc.tensor.matmul(v1[:], lhsT=w1lo_sb[:, 0:128], rhs=h1a[:], start=True, stop=False)
            nc.tensor.matmul(v1[:], lhsT=w1hi_sb[:, 0:128], rhs=h1b[:], start=False, stop=True)
            nc.tensor.matmul(v2[:], lhsT=w1lo_sb[:, 128:H], rhs=h1a[:], start=True, stop=False)
            nc.tensor.matmul(v2[:], lhsT=w1hi_sb[:, 128:H], rhs=h1b[:], start=False, stop=True)
            o1 = opool.tile([P, w], f32, tag="o1")
            o2 = opool.tile([P, w], f32, tag="o2")
            nc.scalar.activation(o1[:], v1[:], mybir.ActivationFunctionType.Relu)
            nc.scalar.activation(o2[:], v2[:], mybir.ActivationFunctionType.Relu)
            nc.sync.dma_start(out_d[0:128, t0 * P:t0 * P + w], o1[:])
            nc.sync.dma_start(out_d[128:H, t0 * P:t0 * P + w], o2[:])

        # phase 1: per-tile gather calls
        n_pieces = [0] * NT
        for (t, c0, w) in calls:
            n_pieces[t] += -(-w // 4)

        def self_only_tile(t):
            acc0 = ps_acc.tile([P, 4 * F], f32, tag="acc")
            nc.tensor.matmul(acc0[:, 0:F], lhsT=ident[:],
                             rhs=selfcols[:, t * F:(t + 1) * F],
                             start=True, stop=True)
            finish_tile(t, acc0, 1)

        cur_t = -1
        accp = None
        nquad = 0
        piece_i = 0
        idx_base = 0
        kq = 0
        for (t, c0, w) in calls:
            if t != cur_t:
                if cur_t >= 0:
                    finish_tile(cur_t, accp, nquad)
                while cur_t + 1 < t:        # tiles with no gather columns
                    cur_t += 1
                    self_only_tile(cur_t)
                cur_t = t
                nquad = min(4, cols_t[t])
                accnew = ps_acc.tile([P, 4 * F], f32, tag="acc")
                accp = accnew
                piece_i = 0
            g = gpool.tile([P, CALL_COLS * F2], bf16, tag="g")
            nidx = w * P
            nc.gpsimd.dma_gather(
                out_ap=_ap3(g[:, :w * F2], w, F2),
                in_ap=ypair_d[:],
                idxs_ap=idx_sb[:, idx_base:idx_base + w * 8],
                num_idxs=nidx, num_idxs_reg=nidx, elem_size=F2,
                single_packet=False, queue_num=kq % NQ)
            kq += 1
            idx_base += w * 8
            gs = spool.tile([P, CALL_COLS * F2], bf16, tag="gs")
            nc.vector.tensor_tensor(
                out=gs[:, :w * F2], in0=g[:, :w * F2],
                in1=norm2_sb[:, 2 * c0:2 * (c0 + w)].to_broadcast([P, 2 * w, F]),
                op=mybir.AluOpType.mult)
            gs2 = s2pool.tile([P, CALL_COLS * F], bf16, tag="gs2")
            ga = gs[:]
            half0 = AP(ga.tensor, ga.offset, [ga.ap[0], [F2, w], [1, F]])
            half1 = AP(ga.tensor, ga.offset + F, [ga.ap[0], [F2, w], [1, F]])
            nc.vector.tensor_tensor(out=gs2[:, :w * F], in0=half0, in1=half1,
                                    op=mybir.AluOpType.add)
            # matmul pieces of up to 4 columns (4-aligned within the tile)
            off = 0
            while off < w:
                pw = min(4, w - off)
                last_data = piece_i == n_pieces[t] - 1
                nc.tensor.matmul(accp[:, :pw * F], lhsT=ident[:],
                                 rhs=gs2[:, off * F:(off + pw) * F],
                                 start=(piece_i == 0),
                                 stop=(last_data and n_pieces[t] > 1))
                if piece_i == 0:
                    # self-loop column rides in quarter 0
                    nc.tensor.matmul(accp[:, 0:F], lhsT=ident[:],
                                     rhs=selfcols[:, t * F:(t + 1) * F],
                                     start=False, stop=(n_pieces[t] == 1))
                piece_i += 1
                off += pw
        if cur_t >= 0:
            finish_tile(cur_t, accp, nquad)
        while cur_t + 1 < NT:
            cur_t += 1
            self_only_tile(cur_t)
    nc.compile()
    return nc


def _run(inputs, trace=False):
    x = np.asarray(inputs["x"])
    W0 = np.asarray(inputs["W0"])
    W1 = np.asarray(inputs["W1"])
    edge_index = np.asarray(inputs["edge_index"])
    in_maps, unshard, meta = _prep(x, W0, W1, edge_index)
    nc = _build(meta)
    res = run_bass_kernel_spmd(nc, in_maps, core_ids=list(range(NCORES)), trace=trace)
    N, H, ND = meta["N"], meta["H"], meta["ND"]
    h = np.empty((N, H), dtype=np.float32)
    for c in range(NCORES):
        o = res.results[c]["out"]            # [H, NT*P]
        nd_c = min(ND, N - c * ND)
        h[c * ND:c * ND + nd_c] = o.T[unshard[c][:nd_c]]
    return h, res


def kernel(**inputs) -> np.ndarray:
    h, _ = _run(inputs, trace=False)
    return h


# revision 10
# speedup vs baseline: 1.3780x; 1.3780x over previous
"""GCN encoder kernel for 8 Trainium2 NeuronCores.

Strategy
--------
out = relu(relu(A_hat @ x @ W0) @ W1), A_hat = D^-1/2 (A + I) D^-1/2.

- Destinations (output rows) are sharded across the 8 cores; each core owns
  N/8 nodes and all edges pointing at them.
- Host-side prep (index work only): per core, edges are bucketed by
  destination, destinations are degree-sorted into tiles of 128, and each
  edge becomes a "slot" (partition = destination's position in its tile,
  column = edge rank).  Slots are gathered from HBM with dma_gather using
  node-PAIR rows (512 B) so the int16 index (= src//2) covers all 50k nodes;
  a per-slot norm pair masks the wanted half.  Per-edge norm
  dinv[src]*dinv[dst] rides in that mask, so the device computes the full
  normalized aggregation.  Self-loop terms skip the gather: the core's own
  x rows arrive position-ordered and are scaled by dinv^2 on device.
- On device: dma_gather over 4 SWDGE queues (the gather is the bottleneck;
  multiple queues overlap ring drain), DVE applies the norm mask and folds
  the pair halves, TensorE accumulates slot columns into PSUM quarters via
  an identity stationary (segment-sum), then the two dense layers run
  feature-major with fused ReLU eviction on ScalarE.
"""

import os
import sys

for _p in ("/opt/trn_rl_repo", "/root/.axon_site/_ro/trn_rl_repo"):
    if os.path.isdir(_p) and _p not in sys.path:
        sys.path.insert(0, _p)

import numpy as np
import ml_dtypes
from contextlib import ExitStack

import concourse.bass as bass
import concourse.tile as tile
from concourse import bacc, mybir
from concourse.bass_utils import run_bass_kernel_spmd
from concourse.masks import make_identity
from concourse.ap import AP

P = 128
NCORES = 8
CALL_COLS = 16         # max slot-columns per dma_gather call (2048 slots)
NQ = 4                 # SWDGE queues
bf16 = mybir.dt.bfloat16
f32 = mybir.dt.float32
i16 = mybir.dt.int16
BF = ml_dtypes.bfloat16


def _ap3(t_ap, d1, d2):
    st = t_ap.ap[-1][0]
    return AP(t_ap.tensor, t_ap.offset, [t_ap.ap[0], [d2 * st, d1], [st, d2]])


def _prep(x, W0, W1, edge_index):
    N, F = x.shape
    H = W0.shape[1]
    ND = (N + NCORES - 1) // NCORES          # dsts per core
    NT = (ND + P - 1) // P                   # dst tiles per core
    NDP = NT * P                             # padded dsts per core

    row = np.asarray(edge_index[0], dtype=np.int64)
    col = np.asarray(edge_index[1], dtype=np.int64)
    deg = np.bincount(col, minlength=N).astype(np.float32) + 1.0
    dinv = (1.0 / np.sqrt(deg)).astype(np.float32)

    norm_e = dinv[row] * dinv[col]
    core_of = col // ND

    npair = (N + 1) // 2 + 1                 # +1 zero pair
    zero_pair = npair - 1
    assert zero_pair <= 32767

    xp = np.zeros((2 * npair, F), dtype=BF)
    xp[:N] = x.astype(BF)
    ypair = xp.reshape(npair, 2 * F)

    per_core = []
    sdeg_tiles = np.zeros((NCORES, NT), dtype=np.int64)
    for c in range(NCORES):
        m = core_of == c
        r = row[m]
        dl = col[m] - c * ND
        nm = norm_e[m]
        key = dl * npair + (r >> 1)
        uniq, inv = np.unique(key, return_inverse=True)
        S0 = uniq.shape[0]
        norm2 = np.zeros((S0, 2), dtype=np.float32)
        np.add.at(norm2, (inv, (r & 1).astype(np.int64)), nm)
        slot_dl = (uniq // npair).astype(np.int64)
        slot_pr = (uniq % npair).astype(np.int64)
        sdeg = np.bincount(slot_dl, minlength=NDP)
        start_of = np.zeros(NDP + 1, dtype=np.int64)
        np.cumsum(sdeg, out=start_of[1:])
        j_rank = np.arange(S0, dtype=np.int64) - start_of[slot_dl]
        perm = np.argsort(-sdeg, kind="stable")       # position -> dst
        pos_of = np.empty(NDP, dtype=np.int64)
        pos_of[perm] = np.arange(NDP)
        sdeg_tiles[c] = sdeg[perm].reshape(NT, P).max(axis=1)
        per_core.append(dict(slot_dl=slot_dl, slot_pr=slot_pr, j_rank=j_rank,
                             norm2=norm2, pos_of=pos_of, perm=perm))

    cols_t = sdeg_tiles.max(axis=0).astype(np.int64)  # ragged, may be 0
    colbase = np.zeros(NT + 1, dtype=np.int64)
    np.cumsum(cols_t, out=colbase[1:])
    C = int(colbase[-1])

    # matmul pieces: <=4 columns, 4-aligned to their tile's first column
    pieces = []  # (tile, col_lo_global, ncols, first_of_tile, last_of_tile)
    for t in range(NT):
        left = int(cols_t[t])
        c0 = int(colbase[t])
        while left > 0:
            w = min(4, left)
            pieces.append([t, c0, w, c0 == int(colbase[t]),
                           left - w == 0])
            c0 += w
            left -= w
    # pack consecutive pieces into gather calls of <= CALL_COLS columns
    calls = []   # (col_lo_global, ncols, [piece indices])
    cur = None
    for pi, (t, c0, w, fo, lo) in enumerate(pieces):
        if cur is None or cur[1] + w > CALL_COLS:
            cur = [c0, 0, []]
            calls.append(cur)
        cur[1] += w
        cur[2].append(pi)

    in_maps = []
    unshard = []
    for c in range(NCORES):
        pc = per_core[c]
        pos = pc["pos_of"][pc["slot_dl"]]
        prow = pos % P
        scol = colbase[pos // P] + pc["j_rank"]
        idx_arr = np.full((P, max(C, 1)), zero_pair, dtype=np.int16)
        idx_arr[prow, scol] = pc["slot_pr"].astype(np.int16)
        norm2_arr = np.zeros((P, 2 * max(C, 1)), dtype=BF)
        norm2_arr[prow, 2 * scol] = pc["norm2"][:, 0].astype(BF)
        norm2_arr[prow, 2 * scol + 1] = pc["norm2"][:, 1].astype(BF)
        # idx re-layout: per call, slot i (= colj*128 + p over the call's
        # columns) lives at [i%16, base*8 + i//16], replicated over 8 row-groups
        blocks = []
        for (c0, w, _ps) in calls:
            blk = idx_arr[:, c0:c0 + w]                       # [128, w]
            v = blk.T.reshape(-1)                             # slot-major
            b = v.reshape(w * 8, 16).T                        # [16, w*8]
            blocks.append(np.tile(b, (8, 1)))
        idx16 = np.concatenate(blocks, axis=1) if blocks else np.zeros((P, 8), np.int16)
        # self-loop inputs: x rows in position order + dinv^2 per position
        nd_c = min(ND, N - c * ND)
        xs = np.zeros((NDP, F), dtype=BF)
        d2 = np.zeros(NDP, dtype=np.float32)
        valid = pc["perm"] < nd_c
        gids = c * ND + pc["perm"][valid]
        xs[valid] = x[gids].astype(BF)
        d2[valid] = dinv[gids] ** 2
        xself = np.ascontiguousarray(
            xs.reshape(NT, P, F).transpose(1, 0, 2).reshape(P, NT * F))
        dinv2 = np.ascontiguousarray(
            d2.reshape(NT, P).T.astype(BF))                   # [128, NT]
        in_maps.append({
            "ypair": ypair,
            "idx": np.ascontiguousarray(idx16),
            "norm2": np.ascontiguousarray(norm2_arr),
            "xself": xself,
            "dinv2": dinv2,
            "w0": W0.astype(BF),
            "w1lo": W1[:128].astype(BF),
            "w1hi": W1[128:].astype(BF),
        })
        unshard.append(pc["pos_of"])

    meta = dict(N=N, F=F, H=H, ND=ND, NT=NT, NDP=NDP, npair=npair,
                C=max(C, 1), cols_t=cols_t.tolist(), colbase=colbase.tolist(),
                calls=calls, pieces=pieces, idx_cols=sum(w * 8 for (_c, w, _ps) in calls))
    return in_maps, unshard, meta


def _build(meta):
    F, H = meta["F"], meta["H"]
    NT, npair = meta["NT"], meta["npair"]
    C, cols_t, colbase = meta["C"], meta["cols_t"], meta["colbase"]
    calls = meta["calls"]
    idx_cols = meta["idx_cols"]
    F2 = 2 * F

    nc = bacc.Bacc(None, target_bir_lowering=False, debug=False,
                   num_devices=NCORES, num_swdge_queues=NQ,
                   dynamic_dma_scratch_size=NQ * CALL_COLS * P * 16)
    ypair_d = nc.declare_dram_parameter("ypair", [npair, F2], bf16, isOutput=False)
    idx_d = nc.declare_dram_parameter("idx", [P, idx_cols], i16, isOutput=False)
    norm2_d = nc.declare_dram_parameter("norm2", [P, 2 * C], bf16, isOutput=False)
    xself_d = nc.declare_dram_parameter("xself", [P, NT * F], bf16, isOutput=False)
    dinv2_d = nc.declare_dram_parameter("dinv2", [P, NT], bf16, isOutput=False)
    w0_d = nc.declare_dram_parameter("w0", [F, H], bf16, isOutput=False)
    w1lo_d = nc.declare_dram_parameter("w1lo", [128, H], bf16, isOutput=False)
    w1hi_d = nc.declare_dram_parameter("w1hi", [H - 128, H], bf16, isOutput=False)
    out_d = nc.declare_dram_parameter("out", [H, NT * P], f32, isOutput=True)

    chunks = [(j * 4, min(4, NT - j * 4)) for j in range((NT + 3) // 4)]

    with tile.TileContext(nc) as tc, ExitStack() as ctx:
        cpool = ctx.enter_context(tc.tile_pool(name="const", bufs=1))
        gpool = ctx.enter_context(tc.tile_pool(name="g", bufs=2))
        spool = ctx.enter_context(tc.tile_pool(name="gs", bufs=2))
        s2pool = ctx.enter_context(tc.tile_pool(name="gs2", bufs=2))
        hpool = ctx.enter_context(tc.tile_pool(name="h0", bufs=2))
        h0Tp = ctx.enter_context(tc.tile_pool(name="h0T", bufs=3))
        h1p = ctx.enter_context(tc.tile_pool(name="h1", bufs=2))
        opool = ctx.enter_context(tc.tile_pool(name="o", bufs=2))
        ps_acc = ctx.enter_context(tc.tile_pool(name="ps_acc", bufs=2, space="PSUM"))
        ps_tr = ctx.enter_context(tc.tile_pool(name="ps_tr", bufs=2, space="PSUM"))
        ps_u = ctx.enter_context(tc.tile_pool(name="ps_u", bufs=1, space="PSUM"))
        ps_v = ctx.enter_context(tc.tile_pool(name="ps_v", bufs=1, space="PSUM"))

        ident = cpool.tile([P, P], bf16)
        make_identity(nc, ident[:])
        # split the index/norm prologue loads so the first gathers start early
        idx_sb = cpool.tile([P, idx_cols], i16)
        n_head = min(idx_cols, 16 * 8 * 4)
        nc.sync.dma_start(idx_sb[:, :n_head], idx_d[:, :n_head])
        if idx_cols > n_head:
            nc.sync.dma_start(idx_sb[:, n_head:], idx_d[:, n_head:])
        norm2_sb = cpool.tile([P, 2 * C], bf16)
        c_head = min(2 * C, 2 * 64)
        nc.sync.dma_start(norm2_sb[:, :c_head], norm2_d[:, :c_head])
        if 2 * C > c_head:
            nc.sync.dma_start(norm2_sb[:, c_head:], norm2_d[:, c_head:])
        xself_sb = cpool.tile([P, NT * F], bf16)
        nc.sync.dma_start(xself_sb[:], xself_d[:])
        dinv2_sb = cpool.tile([P, NT], bf16)
        nc.sync.dma_start(dinv2_sb[:], dinv2_d[:])
        w0_sb = cpool.tile([F, H], bf16)
        nc.sync.dma_start(w0_sb[:], w0_d[:])
        w1lo_sb = cpool.tile([128, H], bf16)
        nc.sync.dma_start(w1lo_sb[:], w1lo_d[:])
        w1hi_sb = cpool.tile([H - 128, H], bf16)
        nc.sync.dma_start(w1hi_sb[:], w1hi_d[:])

        # self-loop columns: xself * dinv2 (broadcast along features), in place
        selfcols = xself_sb
        nc.vector.tensor_tensor(out=selfcols[:], in0=xself_sb[:],
                                in1=dinv2_sb[:].to_broadcast([P, NT, F]),
                                op=mybir.AluOpType.mult)

        h0T_chunk = {}

        def finish_tile(t, accp, nquad):
            h0tmp = hpool.tile([P, P], bf16, tag="h0tmp")
            in_ap = AP(accp[:].tensor, accp[:].offset,
                       [accp[:].ap[0], [1, P], [P, nquad]])
            with nc.allow_low_precision("bf16 h0 evict"):
                nc.vector.tensor_reduce(h0tmp[:], in_ap, axis=mybir.AxisListType.X,
                                        op=mybir.AluOpType.add, opt_input=False)
            trp = ps_tr.tile([P, P], bf16, tag="tr")
            nc.tensor.transpose(trp[:], h0tmp[:], ident[:])
            j = t // 4
            if j not in h0T_chunk:
                w = chunks[j][1] * P
                h0T_new = h0Tp.tile([P, w], bf16, tag="h0T")
                h0T_chunk[j] = h0T_new
            nc.scalar.copy(h0T_chunk[j][:, (t % 4) * P:(t % 4 + 1) * P], trp[:])
            if t % 4 == 3 or t == NT - 1:
                phase2(j)

        def phase2(j):
            t0, ntile = chunks[j]
            w = ntile * P
            h0T = h0T_chunk.pop(j)
            u1 = ps_u.tile([P, w], f32, tag="u1")
            u2 = ps_u.tile([P, w], f32, tag="u2")
            nc.tensor.matmul(u1[:], lhsT=w0_sb[:, 0:128], rhs=h0T[:], start=True, stop=True)
            nc.tensor.matmul(u2[:], lhsT=w0_sb[:, 128:H], rhs=h0T[:], start=True, stop=True)
            h1a = h1p.tile([P, w], bf16, tag="h1a")
            h1b = h1p.tile([P, w], bf16, tag="h1b")
            nc.scalar.activation(h1a[:], u1[:], mybir.ActivationFunctionType.Relu)
            nc.scalar.activation(h1b[:], u2[:], mybir.ActivationFunctionType.Relu)
            v1 = ps_v.tile([P, w], f32, tag="v1")
            v2 = ps_v.tile([P, w], f32, tag="v2")
            nc.tensor.matmul(v1[:], lhsT=w1lo_sb[:, 0:128], rhs=h1a[:], start=True, stop=False)
            nc.tensor.matmul(v1[:], lhsT=w1hi_sb[:, 0:128], rhs=h1b[:], start=False, stop=True)
            nc.tensor.matmul(v2[:], lhsT=w1lo_sb[:, 128:H], rhs=h1a[:], start=True, stop=False)
            nc.tensor.matmul(v2[:], lhsT=w1hi_sb[:, 128:H], rhs=h1b[:], start=False, stop=True)
            o1 = opool.tile([P, w], f32, tag="o1")
            o2 = opool.tile([P, w], f32, tag="o2")
            nc.scalar.activation(o1[:], v1[:], mybir.ActivationFunctionType.Relu)
            nc.scalar.activation(o2[:], v2[:], mybir.ActivationFunctionType.Relu)
            nc.sync.dma_start(out_d[0:128, t0 * P:t0 * P + w], o1[:])
            nc.sync.dma_start(out_d[128:H, t0 * P:t0 * P + w], o2[:])

        # phase 1: piece-packed gather calls
        pieces = meta["pieces"]

        def self_only_tile(t):
            acc0 = ps_acc.tile([P, 4 * F], f32, tag="acc")
            nc.tensor.matmul(acc0[:, 0:F], lhsT=ident[:],
                             rhs=selfcols[:, t * F:(t + 1) * F],
                             start=True, stop=True)
            finish_tile(t, acc0, 1)

        next_tile = 0            # next tile expected to start
        acc_of = {}              # tile -> psum acc
        idx_base = 0
        kq = 0
        for (cstart, cw, plist) in calls:
            g = gpool.tile([P, CALL_COLS * F2], bf16, tag="g")
            nidx = cw * P
            nc.gpsimd.dma_gather(
                out_ap=_ap3(g[:, :cw * F2], cw, F2),
                in_ap=ypair_d[:],
                idxs_ap=idx_sb[:, idx_base:idx_base + cw * 8],
                num_idxs=nidx, num_idxs_reg=nidx, elem_size=F2,
                single_packet=False, queue_num=kq % NQ)
            kq += 1
            idx_base += cw * 8
            gs = spool.tile([P, CALL_COLS * F2], bf16, tag="gs")
            nc.vector.tensor_tensor(
                out=gs[:, :cw * F2], in0=g[:, :cw * F2],
                in1=norm2_sb[:, 2 * cstart:2 * (cstart + cw)]
                    .to_broadcast([P, 2 * cw, F]),
                op=mybir.AluOpType.mult)
            gs2 = s2pool.tile([P, CALL_COLS * F], bf16, tag="gs2")
            ga = gs[:]
            half0 = AP(ga.tensor, ga.offset, [ga.ap[0], [F2, cw], [1, F]])
            half1 = AP(ga.tensor, ga.offset + F, [ga.ap[0], [F2, cw], [1, F]])
            nc.vector.tensor_tensor(out=gs2[:, :cw * F], in0=half0, in1=half1,
                                    op=mybir.AluOpType.add)
            for pi in plist:
                t, c0, pw, first_of_t, last_of_t = pieces[pi]
                if first_of_t:
                    while next_tile < t:     # tiles with no gather columns
                        self_only_tile(next_tile)
                        next_tile += 1
                    accnew = ps_acc.tile([P, 4 * F], f32, tag="acc")
                    acc_of[t] = accnew
                    next_tile = t + 1
                accp = acc_of[t]
                off = c0 - cstart
                nc.tensor.matmul(accp[:, :pw * F], lhsT=ident[:],
                                 rhs=gs2[:, off * F:(off + pw) * F],
                                 start=first_of_t,
                                 stop=(last_of_t and not first_of_t))
                if first_of_t:
                    # self-loop column rides in quarter 0
                    nc.tensor.matmul(accp[:, 0:F], lhsT=ident[:],
                                     rhs=selfcols[:, t * F:(t + 1) * F],
                                     start=False, stop=last_of_t)
                if last_of_t:
                    finish_tile(t, acc_of.pop(t), min(4, cols_t[t]))
        while next_tile < NT:
            self_only_tile(next_tile)
            next_tile += 1
    nc.compile()
    return nc


def _run(inputs, trace=False):
    x = np.asarray(inputs["x"])
    W0 = np.asarray(inputs["W0"])
    W1 = np.asarray(inputs["W1"])
    edge_index = np.asarray(inputs["edge_index"])
    in_maps, unshard, meta = _prep(x, W0, W1, edge_index)
    nc = _build(meta)
    res = run_bass_kernel_spmd(nc, in_maps, core_ids=list(range(NCORES)), trace=trace)
    N, H, ND = meta["N"], meta["H"], meta["ND"]
    h = np.empty((N, H), dtype=np.float32)
    for c in range(NCORES):
        o = res.results[c]["out"]            # [H, NT*P]
        nd_c = min(ND, N - c * ND)
        h[c * ND:c * ND + nd_c] = o.T[unshard[c][:nd_c]]
    return h, res


def kernel(**inputs) -> np.ndarray:
    h, _ = _run(inputs, trace=False)
    return h


# revision 11
# speedup vs baseline: 1.7343x; 1.2586x over previous
"""GCN encoder kernel for 8 Trainium2 NeuronCores.

Strategy
--------
out = relu(relu(A_hat @ x @ W0) @ W1), A_hat = D^-1/2 (A + I) D^-1/2.

- Destinations (output rows) are sharded across the 8 cores; each core owns
  N/8 nodes and all edges pointing at them.
- Host-side prep (index work only): per core, edges are bucketed by
  destination, destinations are degree-sorted into tiles of 128, and each
  edge becomes a "slot" (partition = destination's position in its tile,
  column = edge rank).  Slots are gathered from HBM with dma_gather using
  node-PAIR rows (512 B) so the int16 index (= src//2) covers all 50k nodes;
  a per-slot norm pair masks the wanted half.  Per-edge norm
  dinv[src]*dinv[dst] rides in that mask, so the device computes the full
  normalized aggregation.  Self-loop terms skip the gather: the core's own
  x rows arrive position-ordered and are scaled by dinv^2 on device.
- On device: dma_gather over 4 SWDGE queues (the gather is the bottleneck;
  multiple queues overlap ring drain), DVE applies the norm mask and folds
  the pair halves, TensorE accumulates slot columns into PSUM quarters via
  an identity stationary (segment-sum), then the two dense layers run
  feature-major with fused ReLU eviction on ScalarE.
"""

import os
import sys

for _p in ("/opt/trn_rl_repo", "/root/.axon_site/_ro/trn_rl_repo"):
    if os.path.isdir(_p) and _p not in sys.path:
        sys.path.insert(0, _p)

import numpy as np
import ml_dtypes
from contextlib import ExitStack

import concourse.bass as bass
import concourse.tile as tile
from concourse import bacc, mybir
from concourse.bass_utils import run_bass_kernel_spmd
from concourse.masks import make_identity
from concourse.ap import AP

P = 128
NCORES = 8
CALL_COLS = 16         # max slot-columns per dma_gather call (2048 slots)
NQ = 4                 # SWDGE queues
bf16 = mybir.dt.bfloat16
f32 = mybir.dt.float32
i16 = mybir.dt.int16
BF = ml_dtypes.bfloat16


def _ap3(t_ap, d1, d2):
    st = t_ap.ap[-1][0]
    return AP(t_ap.tensor, t_ap.offset, [t_ap.ap[0], [d2 * st, d1], [st, d2]])


def _prep(x, W0, W1, edge_index):
    N, F = x.shape
    H = W0.shape[1]
    ND = (N + NCORES - 1) // NCORES          # dsts per core
    NT = (ND + P - 1) // P                   # dst tiles per core
    NDP = NT * P                             # padded dsts per core

    row = np.asarray(edge_index[0], dtype=np.int64)
    col = np.asarray(edge_index[1], dtype=np.int64)
    deg = np.bincount(col, minlength=N).astype(np.float32) + 1.0
    dinv = (1.0 / np.sqrt(deg)).astype(np.float32)

    norm_e = dinv[row] * dinv[col]
    core_of = col // ND

    npair = (N + 1) // 2 + 1                 # +1 zero pair
    zero_pair = npair - 1
    assert zero_pair <= 32767

    xp = np.zeros((2 * npair, F), dtype=BF)
    xp[:N] = x.astype(BF)
    ypair = xp.reshape(npair, 2 * F)

    per_core = []
    sdeg_tiles = np.zeros((NCORES, NT), dtype=np.int64)
    for c in range(NCORES):
        m = core_of == c
        r = row[m]
        dl = col[m] - c * ND
        nm = norm_e[m]
        key = dl * npair + (r >> 1)
        uniq, inv = np.unique(key, return_inverse=True)
        S0 = uniq.shape[0]
        norm2 = np.zeros((S0, 2), dtype=np.float32)
        np.add.at(norm2, (inv, (r & 1).astype(np.int64)), nm)
        slot_dl = (uniq // npair).astype(np.int64)
        slot_pr = (uniq % npair).astype(np.int64)
        sdeg = np.bincount(slot_dl, minlength=NDP)
        start_of = np.zeros(NDP + 1, dtype=np.int64)
        np.cumsum(sdeg, out=start_of[1:])
        j_rank = np.arange(S0, dtype=np.int64) - start_of[slot_dl]
        perm = np.argsort(-sdeg, kind="stable")       # position -> dst
        pos_of = np.empty(NDP, dtype=np.int64)
        pos_of[perm] = np.arange(NDP)
        sdeg_tiles[c] = sdeg[perm].reshape(NT, P).max(axis=1)
        per_core.append(dict(slot_dl=slot_dl, slot_pr=slot_pr, j_rank=j_rank,
                             norm2=norm2, pos_of=pos_of, perm=perm))

    cols_t = sdeg_tiles.max(axis=0).astype(np.int64)  # ragged, may be 0
    colbase = np.zeros(NT + 1, dtype=np.int64)
    np.cumsum(cols_t, out=colbase[1:])
    C = int(colbase[-1])

    # matmul pieces: <=4 columns, 4-aligned to their tile's first column
    pieces = []  # (tile, col_lo_global, ncols, first_of_tile, last_of_tile)
    for t in range(NT):
        left = int(cols_t[t])
        c0 = int(colbase[t])
        while left > 0:
            w = min(4, left)
            pieces.append([t, c0, w, c0 == int(colbase[t]),
                           left - w == 0])
            c0 += w
            left -= w
    # pack consecutive pieces into gather calls of <= CALL_COLS columns
    calls = []   # (col_lo_global, ncols, [piece indices])
    cur = None
    for pi, (t, c0, w, fo, lo) in enumerate(pieces):
        if cur is None or cur[1] + w > CALL_COLS:
            cur = [c0, 0, []]
            calls.append(cur)
        cur[1] += w
        cur[2].append(pi)

    in_maps = []
    unshard = []
    for c in range(NCORES):
        pc = per_core[c]
        pos = pc["pos_of"][pc["slot_dl"]]
        prow = pos % P
        scol = colbase[pos // P] + pc["j_rank"]
        idx_arr = np.full((P, max(C, 1)), zero_pair, dtype=np.int16)
        idx_arr[prow, scol] = pc["slot_pr"].astype(np.int16)
        norm2_arr = np.zeros((P, 2 * max(C, 1)), dtype=BF)
        norm2_arr[prow, 2 * scol] = pc["norm2"][:, 0].astype(BF)
        norm2_arr[prow, 2 * scol + 1] = pc["norm2"][:, 1].astype(BF)
        # idx re-layout: per call, slot i (= colj*128 + p over the call's
        # columns) lives at [i%16, base*8 + i//16], replicated over 8 row-groups
        blocks = []
        for (c0, w, _ps) in calls:
            blk = idx_arr[:, c0:c0 + w]                       # [128, w]
            v = blk.T.reshape(-1)                             # slot-major
            b = v.reshape(w * 8, 16).T                        # [16, w*8]
            blocks.append(np.tile(b, (8, 1)))
        idx16 = np.concatenate(blocks, axis=1) if blocks else np.zeros((P, 8), np.int16)
        # self-loop inputs: x rows in position order + dinv^2 per position
        nd_c = min(ND, N - c * ND)
        xs = np.zeros((NDP, F), dtype=BF)
        d2 = np.zeros(NDP, dtype=np.float32)
        valid = pc["perm"] < nd_c
        gids = c * ND + pc["perm"][valid]
        xs[valid] = x[gids].astype(BF)
        d2[valid] = dinv[gids] ** 2
        xself = np.ascontiguousarray(
            xs.reshape(NT, P, F).transpose(1, 0, 2).reshape(P, NT * F))
        dinv2 = np.ascontiguousarray(
            d2.reshape(NT, P).T.astype(BF))                   # [128, NT]
        in_maps.append({
            "ypair": ypair,
            "idx": np.ascontiguousarray(idx16),
            "norm2": np.ascontiguousarray(norm2_arr),
            "xself": xself,
            "dinv2": dinv2,
            "w0": W0.astype(BF),
            "w1lo": W1[:128].astype(BF),
            "w1hi": W1[128:].astype(BF),
        })
        unshard.append(pc["pos_of"])

    meta = dict(N=N, F=F, H=H, ND=ND, NT=NT, NDP=NDP, npair=npair,
                C=max(C, 1), cols_t=cols_t.tolist(), colbase=colbase.tolist(),
                calls=calls, pieces=pieces, idx_cols=sum(w * 8 for (_c, w, _ps) in calls))
    return in_maps, unshard, meta


def _build(meta):
    F, H = meta["F"], meta["H"]
    NT, npair = meta["NT"], meta["npair"]
    C, cols_t, colbase = meta["C"], meta["cols_t"], meta["colbase"]
    calls = meta["calls"]
    idx_cols = meta["idx_cols"]
    F2 = 2 * F

    nc = bacc.Bacc(None, target_bir_lowering=False, debug=False,
                   num_devices=NCORES, num_swdge_queues=NQ,
                   dynamic_dma_scratch_size=NQ * CALL_COLS * P * 16)
    ypair_d = nc.declare_dram_parameter("ypair", [npair, F2], bf16, isOutput=False)
    idx_d = nc.declare_dram_parameter("idx", [P, idx_cols], i16, isOutput=False)
    norm2_d = nc.declare_dram_parameter("norm2", [P, 2 * C], bf16, isOutput=False)
    xself_d = nc.declare_dram_parameter("xself", [P, NT * F], bf16, isOutput=False)
    dinv2_d = nc.declare_dram_parameter("dinv2", [P, NT], bf16, isOutput=False)
    w0_d = nc.declare_dram_parameter("w0", [F, H], bf16, isOutput=False)
    w1lo_d = nc.declare_dram_parameter("w1lo", [128, H], bf16, isOutput=False)
    w1hi_d = nc.declare_dram_parameter("w1hi", [H - 128, H], bf16, isOutput=False)
    out_d = nc.declare_dram_parameter("out", [H, NT * P], f32, isOutput=True)

    chunks = [(j * 4, min(4, NT - j * 4)) for j in range((NT + 3) // 4)]

    with tile.TileContext(nc) as tc, ExitStack() as ctx:
        cpool = ctx.enter_context(tc.tile_pool(name="const", bufs=1))
        gpool = ctx.enter_context(tc.tile_pool(name="g", bufs=3))
        spool = ctx.enter_context(tc.tile_pool(name="gs", bufs=2))
        s2pool = ctx.enter_context(tc.tile_pool(name="gs2", bufs=2))
        hpool = ctx.enter_context(tc.tile_pool(name="h0", bufs=2))
        h0Tp = ctx.enter_context(tc.tile_pool(name="h0T", bufs=3))
        h1p = ctx.enter_context(tc.tile_pool(name="h1", bufs=2))
        opool = ctx.enter_context(tc.tile_pool(name="o", bufs=2))
        ps_acc = ctx.enter_context(tc.tile_pool(name="ps_acc", bufs=2, space="PSUM"))
        ps_tr = ctx.enter_context(tc.tile_pool(name="ps_tr", bufs=2, space="PSUM"))
        ps_u = ctx.enter_context(tc.tile_pool(name="ps_u", bufs=1, space="PSUM"))
        ps_v = ctx.enter_context(tc.tile_pool(name="ps_v", bufs=1, space="PSUM"))

        ident = cpool.tile([P, P], bf16)
        make_identity(nc, ident[:])
        # split the index/norm prologue loads so the first gathers start early
        idx_sb = cpool.tile([P, idx_cols], i16)
        n_head = min(idx_cols, 16 * 8 * 4)
        nc.sync.dma_start(idx_sb[:, :n_head], idx_d[:, :n_head])
        if idx_cols > n_head:
            nc.sync.dma_start(idx_sb[:, n_head:], idx_d[:, n_head:])
        norm2_sb = cpool.tile([P, 2 * C], bf16)
        c_head = min(2 * C, 2 * 64)
        nc.sync.dma_start(norm2_sb[:, :c_head], norm2_d[:, :c_head])
        if 2 * C > c_head:
            nc.sync.dma_start(norm2_sb[:, c_head:], norm2_d[:, c_head:])
        xself_sb = cpool.tile([P, NT * F], bf16)
        nc.sync.dma_start(xself_sb[:], xself_d[:])
        dinv2_sb = cpool.tile([P, NT], bf16)
        nc.sync.dma_start(dinv2_sb[:], dinv2_d[:])
        w0_sb = cpool.tile([F, H], bf16)
        nc.sync.dma_start(w0_sb[:], w0_d[:])
        w1lo_sb = cpool.tile([128, H], bf16)
        nc.sync.dma_start(w1lo_sb[:], w1lo_d[:])
        w1hi_sb = cpool.tile([H - 128, H], bf16)
        nc.sync.dma_start(w1hi_sb[:], w1hi_d[:])

        # self-loop columns: xself * dinv2 (broadcast along features), in place
        selfcols = xself_sb
        nc.vector.tensor_tensor(out=selfcols[:], in0=xself_sb[:],
                                in1=dinv2_sb[:].to_broadcast([P, NT, F]),
                                op=mybir.AluOpType.mult)

        h0T_chunk = {}

        def finish_tile(t, accp, nquad):
            h0tmp = hpool.tile([P, P], bf16, tag="h0tmp")
            in_ap = AP(accp[:].tensor, accp[:].offset,
                       [accp[:].ap[0], [1, P], [P, nquad]])
            with nc.allow_low_precision("bf16 h0 evict"):
                nc.vector.tensor_reduce(h0tmp[:], in_ap, axis=mybir.AxisListType.X,
                                        op=mybir.AluOpType.add, opt_input=False)
            trp = ps_tr.tile([P, P], bf16, tag="tr")
            nc.tensor.transpose(trp[:], h0tmp[:], ident[:])
            j = t // 4
            if j not in h0T_chunk:
                w = chunks[j][1] * P
                h0T_new = h0Tp.tile([P, w], bf16, tag="h0T")
                h0T_chunk[j] = h0T_new
            nc.scalar.copy(h0T_chunk[j][:, (t % 4) * P:(t % 4 + 1) * P], trp[:])
            if t % 4 == 3 or t == NT - 1:
                phase2(j)

        def phase2(j):
            t0, ntile = chunks[j]
            w = ntile * P
            h0T = h0T_chunk.pop(j)
            u1 = ps_u.tile([P, w], f32, tag="u1")
            u2 = ps_u.tile([P, w], f32, tag="u2")
            nc.tensor.matmul(u1[:], lhsT=w0_sb[:, 0:128], rhs=h0T[:], start=True, stop=True)
            nc.tensor.matmul(u2[:], lhsT=w0_sb[:, 128:H], rhs=h0T[:], start=True, stop=True)
            h1a = h1p.tile([P, w], bf16, tag="h1a")
            h1b = h1p.tile([P, w], bf16, tag="h1b")
            nc.scalar.activation(h1a[:], u1[:], mybir.ActivationFunctionType.Relu)
            nc.scalar.activation(h1b[:], u2[:], mybir.ActivationFunctionType.Relu)
            v1 = ps_v.tile([P, w], f32, tag="v1")
            v2 = ps_v.tile([P, w], f32, tag="v2")
            nc.tensor.matmul(v1[:], lhsT=w1lo_sb[:, 0:128], rhs=h1a[:], start=True, stop=False)
            nc.tensor.matmul(v1[:], lhsT=w1hi_sb[:, 0:128], rhs=h1b[:], start=False, stop=True)
            nc.tensor.matmul(v2[:], lhsT=w1lo_sb[:, 128:H], rhs=h1a[:], start=True, stop=False)
            nc.tensor.matmul(v2[:], lhsT=w1hi_sb[:, 128:H], rhs=h1b[:], start=False, stop=True)
            o1 = opool.tile([P, w], f32, tag="o1")
            o2 = opool.tile([P, w], f32, tag="o2")
            nc.scalar.activation(o1[:], v1[:], mybir.ActivationFunctionType.Relu)
            nc.scalar.activation(o2[:], v2[:], mybir.ActivationFunctionType.Relu)
            nc.sync.dma_start(out_d[0:128, t0 * P:t0 * P + w], o1[:])
            nc.sync.dma_start(out_d[128:H, t0 * P:t0 * P + w], o2[:])

        # phase 1: piece-packed gather calls
        pieces = meta["pieces"]

        def self_only_tile(t):
            acc0 = ps_acc.tile([P, 4 * F], f32, tag="acc")
            nc.tensor.matmul(acc0[:, 0:F], lhsT=ident[:],
                             rhs=selfcols[:, t * F:(t + 1) * F],
                             start=True, stop=True)
            finish_tile(t, acc0, 1)

        next_tile = 0            # next tile expected to start
        acc_of = {}              # tile -> psum acc
        idx_base = 0
        kq = 0
        for (cstart, cw, plist) in calls:
            g = gpool.tile([P, CALL_COLS * F2], bf16, tag="g")
            nidx = cw * P
            nc.gpsimd.dma_gather(
                out_ap=_ap3(g[:, :cw * F2], cw, F2),
                in_ap=ypair_d[:],
                idxs_ap=idx_sb[:, idx_base:idx_base + cw * 8],
                num_idxs=nidx, num_idxs_reg=nidx, elem_size=F2,
                single_packet=False, queue_num=kq % NQ)
            kq += 1
            idx_base += cw * 8
            gs = spool.tile([P, CALL_COLS * F2], bf16, tag="gs")
            nc.vector.tensor_tensor(
                out=gs[:, :cw * F2], in0=g[:, :cw * F2],
                in1=norm2_sb[:, 2 * cstart:2 * (cstart + cw)]
                    .to_broadcast([P, 2 * cw, F]),
                op=mybir.AluOpType.mult)
            gs2 = s2pool.tile([P, CALL_COLS * F], bf16, tag="gs2")
            ga = gs[:]
            half0 = AP(ga.tensor, ga.offset, [ga.ap[0], [F2, cw], [1, F]])
            half1 = AP(ga.tensor, ga.offset + F, [ga.ap[0], [F2, cw], [1, F]])
            nc.vector.tensor_tensor(out=gs2[:, :cw * F], in0=half0, in1=half1,
                                    op=mybir.AluOpType.add)
            for pi in plist:
                t, c0, pw, first_of_t, last_of_t = pieces[pi]
                if first_of_t:
                    while next_tile < t:     # tiles with no gather columns
                        self_only_tile(next_tile)
                        next_tile += 1
                    accnew = ps_acc.tile([P, 4 * F], f32, tag="acc")
                    acc_of[t] = accnew
                    next_tile = t + 1
                accp = acc_of[t]
                off = c0 - cstart
                nc.tensor.matmul(accp[:, :pw * F], lhsT=ident[:],
                                 rhs=gs2[:, off * F:(off + pw) * F],
                                 start=first_of_t,
                                 stop=(last_of_t and not first_of_t))
                if first_of_t:
                    # self-loop column rides in quarter 0
                    nc.tensor.matmul(accp[:, 0:F], lhsT=ident[:],
                                     rhs=selfcols[:, t * F:(t + 1) * F],
                                     start=False, stop=last_of_t)
                if last_of_t:
                    finish_tile(t, acc_of.pop(t), min(4, cols_t[t]))
        while next_tile < NT:
            self_only_tile(next_tile)
            next_tile += 1
    nc.compile()
    return nc


def _run(inputs, trace=False):
    x = np.asarray(inputs["x"])
    W0 = np.asarray(inputs["W0"])
    W1 = np.asarray(inputs["W1"])
    edge_index = np.asarray(inputs["edge_index"])
    in_maps, unshard, meta = _prep(x, W0, W1, edge_index)
    nc = _build(meta)
    res = run_bass_kernel_spmd(nc, in_maps, core_ids=list(range(NCORES)), trace=trace)
    N, H, ND = meta["N"], meta["H"], meta["ND"]
    h = np.empty((N, H), dtype=np.float32)
    for c in range(NCORES):
        o = res.results[c]["out"]            # [H, NT*P]
        nd_c = min(ND, N - c * ND)
        h[c * ND:c * ND + nd_c] = o.T[unshard[c][:nd_c]]
    return h, res


def kernel(**inputs) -> np.ndarray:
    h, _ = _run(inputs, trace=False)
    return h


# revision 12
# speedup vs baseline: 1.8802x; 1.0841x over previous
"""GCN encoder kernel for 8 Trainium2 NeuronCores.

Strategy
--------
out = relu(relu(A_hat @ x @ W0) @ W1), A_hat = D^-1/2 (A + I) D^-1/2.

- Destinations (output rows) are sharded across the 8 cores; each core owns
  N/8 nodes and all edges pointing at them.
- Host-side prep (index work only): per core, edges are bucketed by
  destination, destinations are degree-sorted into tiles of 128, and each
  edge becomes a "slot" (partition = destination's position in its tile,
  column = edge rank).  Slots are gathered from HBM with dma_gather using
  node-PAIR rows (512 B) so the int16 index (= src//2) covers all 50k nodes;
  a per-slot norm pair masks the wanted half.  Per-edge norm
  dinv[src]*dinv[dst] rides in that mask, so the device computes the full
  normalized aggregation.  Self-loop terms skip the gather: the core's own
  x rows arrive position-ordered and are scaled by dinv^2 on device.
- On device: dma_gather over 4 SWDGE queues (the gather is the bottleneck;
  multiple queues overlap ring drain), DVE applies the norm mask and folds
  the pair halves, TensorE accumulates slot columns into PSUM quarters via
  an identity stationary (segment-sum), then the two dense layers run
  feature-major with fused ReLU eviction on ScalarE.
"""

import os
import sys

for _p in ("/opt/trn_rl_repo", "/root/.axon_site/_ro/trn_rl_repo"):
    if os.path.isdir(_p) and _p not in sys.path:
        sys.path.insert(0, _p)

import numpy as np
import ml_dtypes
from contextlib import ExitStack

import concourse.bass as bass
import concourse.tile as tile
from concourse import bacc, mybir
from concourse.bass_utils import run_bass_kernel_spmd
from concourse.masks import make_identity
from concourse.ap import AP

P = 128
NCORES = 8
CALL_COLS = 16         # max slot-columns per dma_gather call (2048 slots)
NQ = 4                 # SWDGE queues
bf16 = mybir.dt.bfloat16
f32 = mybir.dt.float32
i16 = mybir.dt.int16
BF = ml_dtypes.bfloat16


def _ap3(t_ap, d1, d2):
    st = t_ap.ap[-1][0]
    return AP(t_ap.tensor, t_ap.offset, [t_ap.ap[0], [d2 * st, d1], [st, d2]])


def _prep(x, W0, W1, edge_index):
    N, F = x.shape
    H = W0.shape[1]
    ND = (N + NCORES - 1) // NCORES          # dsts per core
    NT = (ND + P - 1) // P                   # dst tiles per core
    NDP = NT * P                             # padded dsts per core

    row = np.asarray(edge_index[0], dtype=np.int64)
    col = np.asarray(edge_index[1], dtype=np.int64)
    deg = np.bincount(col, minlength=N).astype(np.float32) + 1.0
    dinv = (1.0 / np.sqrt(deg)).astype(np.float32)

    norm_e = dinv[row] * dinv[col]
    core_of = col // ND

    npair = (N + 1) // 2 + 1                 # +1 zero pair
    zero_pair = npair - 1
    assert zero_pair <= 32767

    xp = np.zeros((2 * npair, F), dtype=BF)
    xp[:N] = x.astype(BF)
    ypair = xp.reshape(npair, 2 * F)

    per_core = []
    sdeg_tiles = np.zeros((NCORES, NT), dtype=np.int64)
    for c in range(NCORES):
        m = core_of == c
        r = row[m]
        dl = col[m] - c * ND
        nm = norm_e[m]
        key = dl * npair + (r >> 1)
        uniq, inv = np.unique(key, return_inverse=True)
        S0 = uniq.shape[0]
        norm2 = np.zeros((S0, 2), dtype=np.float32)
        np.add.at(norm2, (inv, (r & 1).astype(np.int64)), nm)
        slot_dl = (uniq // npair).astype(np.int64)
        slot_pr = (uniq % npair).astype(np.int64)
        sdeg = np.bincount(slot_dl, minlength=NDP)
        start_of = np.zeros(NDP + 1, dtype=np.int64)
        np.cumsum(sdeg, out=start_of[1:])
        j_rank = np.arange(S0, dtype=np.int64) - start_of[slot_dl]
        perm = np.argsort(-sdeg, kind="stable")       # position -> dst
        pos_of = np.empty(NDP, dtype=np.int64)
        pos_of[perm] = np.arange(NDP)
        sdeg_tiles[c] = sdeg[perm].reshape(NT, P).max(axis=1)
        per_core.append(dict(slot_dl=slot_dl, slot_pr=slot_pr, j_rank=j_rank,
                             norm2=norm2, pos_of=pos_of, perm=perm))

    cols_t = sdeg_tiles.max(axis=0).astype(np.int64)  # ragged, may be 0
    colbase = np.zeros(NT + 1, dtype=np.int64)
    np.cumsum(cols_t, out=colbase[1:])
    C = int(colbase[-1])

    # matmul pieces: <=2 pair-columns, 2-aligned to their tile's first column
    pieces = []  # (tile, col_lo_global, ncols, first_of_tile, last_of_tile)
    for t in range(NT):
        left = int(cols_t[t])
        c0 = int(colbase[t])
        while left > 0:
            w = min(2, left)
            pieces.append([t, c0, w, c0 == int(colbase[t]),
                           left - w == 0])
            c0 += w
            left -= w
    # pack consecutive pieces into gather calls of <= CALL_COLS columns
    calls = []   # (col_lo_global, ncols, [piece indices])
    cur = None
    for pi, (t, c0, w, fo, lo) in enumerate(pieces):
        if cur is None or cur[1] + w > CALL_COLS:
            cur = [c0, 0, []]
            calls.append(cur)
        cur[1] += w
        cur[2].append(pi)

    in_maps = []
    unshard = []
    for c in range(NCORES):
        pc = per_core[c]
        pos = pc["pos_of"][pc["slot_dl"]]
        prow = pos % P
        scol = colbase[pos // P] + pc["j_rank"]
        idx_arr = np.full((P, max(C, 1)), zero_pair, dtype=np.int16)
        idx_arr[prow, scol] = pc["slot_pr"].astype(np.int16)
        norm2_arr = np.zeros((P, 2 * max(C, 1)), dtype=BF)
        norm2_arr[prow, 2 * scol] = pc["norm2"][:, 0].astype(BF)
        norm2_arr[prow, 2 * scol + 1] = pc["norm2"][:, 1].astype(BF)
        # idx re-layout: per call, slot i (= colj*128 + p over the call's
        # columns) lives at [i%16, base*8 + i//16], replicated over 8 row-groups
        blocks = []
        for (c0, w, _ps) in calls:
            blk = idx_arr[:, c0:c0 + w]                       # [128, w]
            v = blk.T.reshape(-1)                             # slot-major
            b = v.reshape(w * 8, 16).T                        # [16, w*8]
            blocks.append(np.tile(b, (8, 1)))
        idx16 = np.concatenate(blocks, axis=1) if blocks else np.zeros((P, 8), np.int16)
        # self-loop inputs: x rows in position order + dinv^2 per position
        nd_c = min(ND, N - c * ND)
        xs = np.zeros((NDP, F), dtype=BF)
        d2 = np.zeros(NDP, dtype=np.float32)
        valid = pc["perm"] < nd_c
        gids = c * ND + pc["perm"][valid]
        xs[valid] = x[gids].astype(BF)
        d2[valid] = dinv[gids] ** 2
        xself = np.ascontiguousarray(
            xs.reshape(NT, P, F).transpose(1, 0, 2).reshape(P, NT * F))
        dinv2 = np.ascontiguousarray(
            d2.reshape(NT, P).T.astype(BF))                   # [128, NT]
        in_maps.append({
            "ypair": ypair,
            "idx": np.ascontiguousarray(idx16),
            "norm2": np.ascontiguousarray(norm2_arr),
            "xself": xself,
            "dinv2": dinv2,
            "w0": W0.astype(BF),
            "w1lo": W1[:128].astype(BF),
            "w1hi": W1[128:].astype(BF),
        })
        unshard.append(pc["pos_of"])

    meta = dict(N=N, F=F, H=H, ND=ND, NT=NT, NDP=NDP, npair=npair,
                C=max(C, 1), cols_t=cols_t.tolist(), colbase=colbase.tolist(),
                calls=calls, pieces=pieces, idx_cols=sum(w * 8 for (_c, w, _ps) in calls))
    return in_maps, unshard, meta


def _build(meta):
    F, H = meta["F"], meta["H"]
    NT, npair = meta["NT"], meta["npair"]
    C, cols_t, colbase = meta["C"], meta["cols_t"], meta["colbase"]
    calls = meta["calls"]
    idx_cols = meta["idx_cols"]
    F2 = 2 * F

    nc = bacc.Bacc(None, target_bir_lowering=False, debug=False,
                   num_devices=NCORES, num_swdge_queues=NQ,
                   dynamic_dma_scratch_size=NQ * CALL_COLS * P * 16)
    ypair_d = nc.declare_dram_parameter("ypair", [npair, F2], bf16, isOutput=False)
    idx_d = nc.declare_dram_parameter("idx", [P, idx_cols], i16, isOutput=False)
    norm2_d = nc.declare_dram_parameter("norm2", [P, 2 * C], bf16, isOutput=False)
    xself_d = nc.declare_dram_parameter("xself", [P, NT * F], bf16, isOutput=False)
    dinv2_d = nc.declare_dram_parameter("dinv2", [P, NT], bf16, isOutput=False)
    w0_d = nc.declare_dram_parameter("w0", [F, H], bf16, isOutput=False)
    w1lo_d = nc.declare_dram_parameter("w1lo", [128, H], bf16, isOutput=False)
    w1hi_d = nc.declare_dram_parameter("w1hi", [H - 128, H], bf16, isOutput=False)
    out_d = nc.declare_dram_parameter("out", [H, NT * P], f32, isOutput=True)

    chunks = [(j * 4, min(4, NT - j * 4)) for j in range((NT + 3) // 4)]

    with tile.TileContext(nc) as tc, ExitStack() as ctx:
        cpool = ctx.enter_context(tc.tile_pool(name="const", bufs=1))
        gpool = ctx.enter_context(tc.tile_pool(name="g", bufs=3))
        spool = ctx.enter_context(tc.tile_pool(name="gs", bufs=3))
        hpool = ctx.enter_context(tc.tile_pool(name="h0", bufs=2))
        h0Tp = ctx.enter_context(tc.tile_pool(name="h0T", bufs=3))
        h1p = ctx.enter_context(tc.tile_pool(name="h1", bufs=2))
        opool = ctx.enter_context(tc.tile_pool(name="o", bufs=2))
        ps_acc = ctx.enter_context(tc.tile_pool(name="ps_acc", bufs=2, space="PSUM"))
        ps_tr = ctx.enter_context(tc.tile_pool(name="ps_tr", bufs=2, space="PSUM"))
        ps_u = ctx.enter_context(tc.tile_pool(name="ps_u", bufs=1, space="PSUM"))
        ps_v = ctx.enter_context(tc.tile_pool(name="ps_v", bufs=1, space="PSUM"))

        ident = cpool.tile([P, P], bf16)
        make_identity(nc, ident[:])
        # split the index/norm prologue loads so the first gathers start early
        idx_sb = cpool.tile([P, idx_cols], i16)
        n_head = min(idx_cols, 16 * 8 * 4)
        nc.sync.dma_start(idx_sb[:, :n_head], idx_d[:, :n_head])
        if idx_cols > n_head:
            nc.sync.dma_start(idx_sb[:, n_head:], idx_d[:, n_head:])
        norm2_sb = cpool.tile([P, 2 * C], bf16)
        c_head = min(2 * C, 2 * 64)
        nc.sync.dma_start(norm2_sb[:, :c_head], norm2_d[:, :c_head])
        if 2 * C > c_head:
            nc.sync.dma_start(norm2_sb[:, c_head:], norm2_d[:, c_head:])
        xself_sb = cpool.tile([P, NT * F], bf16)
        nc.sync.dma_start(xself_sb[:], xself_d[:])
        dinv2_sb = cpool.tile([P, NT], bf16)
        nc.sync.dma_start(dinv2_sb[:], dinv2_d[:])
        w0_sb = cpool.tile([F, H], bf16)
        nc.sync.dma_start(w0_sb[:], w0_d[:])
        w1lo_sb = cpool.tile([128, H], bf16)
        nc.sync.dma_start(w1lo_sb[:], w1lo_d[:])
        w1hi_sb = cpool.tile([H - 128, H], bf16)
        nc.sync.dma_start(w1hi_sb[:], w1hi_d[:])

        # self-loop columns: xself * dinv2 (broadcast along features), in place
        selfcols = xself_sb
        nc.vector.tensor_tensor(out=selfcols[:], in0=xself_sb[:],
                                in1=dinv2_sb[:].to_broadcast([P, NT, F]),
                                op=mybir.AluOpType.mult)

        h0T_chunk = {}

        def finish_tile(t, accp, nquad):
            h0tmp = hpool.tile([P, P], bf16, tag="h0tmp")
            in_ap = AP(accp[:].tensor, accp[:].offset,
                       [accp[:].ap[0], [1, P], [P, nquad]])
            with nc.allow_low_precision("bf16 h0 evict"):
                nc.vector.tensor_reduce(h0tmp[:], in_ap, axis=mybir.AxisListType.X,
                                        op=mybir.AluOpType.add, opt_input=False)
            trp = ps_tr.tile([P, P], bf16, tag="tr")
            nc.tensor.transpose(trp[:], h0tmp[:], ident[:])
            j = t // 4
            if j not in h0T_chunk:
                w = chunks[j][1] * P
                h0T_new = h0Tp.tile([P, w], bf16, tag="h0T")
                h0T_chunk[j] = h0T_new
            nc.scalar.copy(h0T_chunk[j][:, (t % 4) * P:(t % 4 + 1) * P], trp[:])
            if t % 4 == 3 or t == NT - 1:
                phase2(j)

        def phase2(j):
            t0, ntile = chunks[j]
            w = ntile * P
            h0T = h0T_chunk.pop(j)
            u1 = ps_u.tile([P, w], f32, tag="u1")
            u2 = ps_u.tile([P, w], f32, tag="u2")
            nc.tensor.matmul(u1[:], lhsT=w0_sb[:, 0:128], rhs=h0T[:], start=True, stop=True)
            nc.tensor.matmul(u2[:], lhsT=w0_sb[:, 128:H], rhs=h0T[:], start=True, stop=True)
            h1a = h1p.tile([P, w], bf16, tag="h1a")
            h1b = h1p.tile([P, w], bf16, tag="h1b")
            nc.scalar.activation(h1a[:], u1[:], mybir.ActivationFunctionType.Relu)
            nc.scalar.activation(h1b[:], u2[:], mybir.ActivationFunctionType.Relu)
            v1 = ps_v.tile([P, w], f32, tag="v1")
            v2 = ps_v.tile([P, w], f32, tag="v2")
            nc.tensor.matmul(v1[:], lhsT=w1lo_sb[:, 0:128], rhs=h1a[:], start=True, stop=False)
            nc.tensor.matmul(v1[:], lhsT=w1hi_sb[:, 0:128], rhs=h1b[:], start=False, stop=True)
            nc.tensor.matmul(v2[:], lhsT=w1lo_sb[:, 128:H], rhs=h1a[:], start=True, stop=False)
            nc.tensor.matmul(v2[:], lhsT=w1hi_sb[:, 128:H], rhs=h1b[:], start=False, stop=True)
            o1 = opool.tile([P, w], f32, tag="o1")
            o2 = opool.tile([P, w], f32, tag="o2")
            nc.scalar.activation(o1[:], v1[:], mybir.ActivationFunctionType.Relu)
            nc.scalar.activation(o2[:], v2[:], mybir.ActivationFunctionType.Relu)
            nc.sync.dma_start(out_d[0:128, t0 * P:t0 * P + w], o1[:])
            nc.sync.dma_start(out_d[128:H, t0 * P:t0 * P + w], o2[:])

        # phase 1: piece-packed gather calls
        pieces = meta["pieces"]

        def self_only_tile(t):
            acc0 = ps_acc.tile([P, 4 * F], f32, tag="acc")
            nc.tensor.matmul(acc0[:, 0:F], lhsT=ident[:],
                             rhs=selfcols[:, t * F:(t + 1) * F],
                             start=True, stop=True)
            finish_tile(t, acc0, 1)

        next_tile = 0            # next tile expected to start
        acc_of = {}              # tile -> psum acc
        idx_base = 0
        kq = 0
        for (cstart, cw, plist) in calls:
            g = gpool.tile([P, CALL_COLS * F2], bf16, tag="g")
            nidx = cw * P
            nc.gpsimd.dma_gather(
                out_ap=_ap3(g[:, :cw * F2], cw, F2),
                in_ap=ypair_d[:],
                idxs_ap=idx_sb[:, idx_base:idx_base + cw * 8],
                num_idxs=nidx, num_idxs_reg=nidx, elem_size=F2,
                single_packet=False, queue_num=kq % NQ)
            kq += 1
            idx_base += cw * 8
            gs = spool.tile([P, CALL_COLS * F2], bf16, tag="gs")
            nc.vector.tensor_tensor(
                out=gs[:, :cw * F2], in0=g[:, :cw * F2],
                in1=norm2_sb[:, 2 * cstart:2 * (cstart + cw)]
                    .to_broadcast([P, 2 * cw, F]),
                op=mybir.AluOpType.mult)
            for pi in plist:
                t, c0, pw, first_of_t, last_of_t = pieces[pi]
                if first_of_t:
                    while next_tile < t:     # tiles with no gather columns
                        self_only_tile(next_tile)
                        next_tile += 1
                    accnew = ps_acc.tile([P, 4 * F], f32, tag="acc")
                    acc_of[t] = accnew
                    next_tile = t + 1
                accp = acc_of[t]
                off = c0 - cstart
                nc.tensor.matmul(accp[:, :pw * F2], lhsT=ident[:],
                                 rhs=gs[:, off * F2:(off + pw) * F2],
                                 start=first_of_t,
                                 stop=(last_of_t and not first_of_t))
                if first_of_t:
                    # self-loop column rides in quarter 0
                    nc.tensor.matmul(accp[:, 0:F], lhsT=ident[:],
                                     rhs=selfcols[:, t * F:(t + 1) * F],
                                     start=False, stop=last_of_t)
                if last_of_t:
                    finish_tile(t, acc_of.pop(t), min(4, 2 * cols_t[t]))
        while next_tile < NT:
            self_only_tile(next_tile)
            next_tile += 1
    nc.compile()
    return nc


def _run(inputs, trace=False):
    x = np.asarray(inputs["x"])
    W0 = np.asarray(inputs["W0"])
    W1 = np.asarray(inputs["W1"])
    edge_index = np.asarray(inputs["edge_index"])
    in_maps, unshard, meta = _prep(x, W0, W1, edge_index)
    nc = _build(meta)
    res = run_bass_kernel_spmd(nc, in_maps, core_ids=list(range(NCORES)), trace=trace)
    N, H, ND = meta["N"], meta["H"], meta["ND"]
    h = np.empty((N, H), dtype=np.float32)
    for c in range(NCORES):
        o = res.results[c]["out"]            # [H, NT*P]
        nd_c = min(ND, N - c * ND)
        h[c * ND:c * ND + nd_c] = o.T[unshard[c][:nd_c]]
    return h, res


def kernel(**inputs) -> np.ndarray:
    h, _ = _run(inputs, trace=False)
    return h


# revision 13
# speedup vs baseline: 1.9577x; 1.0412x over previous
"""GCN encoder kernel for 8 Trainium2 NeuronCores.

Strategy
--------
out = relu(relu(A_hat @ x @ W0) @ W1), A_hat = D^-1/2 (A + I) D^-1/2.

- Destinations (output rows) are sharded across the 8 cores; each core owns
  N/8 nodes and all edges pointing at them.
- Host-side prep (index work only): per core, edges are bucketed by
  destination, destinations are degree-sorted into tiles of 128, and each
  edge becomes a "slot" (partition = destination's position in its tile,
  column = edge rank).  Slots are gathered from HBM with dma_gather using
  node-PAIR rows (512 B) so the int16 index (= src//2) covers all 50k nodes;
  a per-slot norm pair masks the wanted half.  Per-edge norm
  dinv[src]*dinv[dst] rides in that mask, so the device computes the full
  normalized aggregation.  Self-loop terms skip the gather: the core's own
  x rows arrive position-ordered and are scaled by dinv^2 on device.
- On device: dma_gather over 4 SWDGE queues (the gather is the bottleneck;
  multiple queues overlap ring drain), DVE applies the norm mask and folds
  the pair halves, TensorE accumulates slot columns into PSUM quarters via
  an identity stationary (segment-sum), then the two dense layers run
  feature-major with fused ReLU eviction on ScalarE.
"""

import os
import sys

for _p in ("/opt/trn_rl_repo", "/root/.axon_site/_ro/trn_rl_repo"):
    if os.path.isdir(_p) and _p not in sys.path:
        sys.path.insert(0, _p)

import numpy as np
import ml_dtypes
from contextlib import ExitStack

import concourse.bass as bass
import concourse.tile as tile
from concourse import bacc, mybir
from concourse.bass_utils import run_bass_kernel_spmd
from concourse.ap import AP

P = 128
NCORES = 8
CALL_COLS = 16         # max slot-columns per dma_gather call (2048 slots)
NQ = 4                 # SWDGE queues
bf16 = mybir.dt.bfloat16
f32 = mybir.dt.float32
i16 = mybir.dt.int16
BF = ml_dtypes.bfloat16


def _ap3(t_ap, d1, d2):
    st = t_ap.ap[-1][0]
    return AP(t_ap.tensor, t_ap.offset, [t_ap.ap[0], [d2 * st, d1], [st, d2]])


def _prep(x, W0, W1, edge_index):
    N, F = x.shape
    H = W0.shape[1]
    ND = (N + NCORES - 1) // NCORES          # dsts per core
    NT = (ND + P - 1) // P                   # dst tiles per core
    NDP = NT * P                             # padded dsts per core

    row = np.asarray(edge_index[0], dtype=np.int64)
    col = np.asarray(edge_index[1], dtype=np.int64)
    deg = np.bincount(col, minlength=N).astype(np.float32) + 1.0
    dinv = (1.0 / np.sqrt(deg)).astype(np.float32)

    norm_e = dinv[row] * dinv[col]
    core_of = col // ND

    npair = (N + 1) // 2 + 1                 # +1 zero pair
    zero_pair = npair - 1
    assert zero_pair <= 32767

    xp = np.zeros((2 * npair, F), dtype=BF)
    xp[:N] = x.astype(BF)
    ypair = xp.reshape(npair, 2 * F)

    per_core = []
    sdeg_tiles = np.zeros((NCORES, NT), dtype=np.int64)
    for c in range(NCORES):
        m = core_of == c
        r = row[m]
        dl = col[m] - c * ND
        nm = norm_e[m]
        key = dl * npair + (r >> 1)
        uniq, inv = np.unique(key, return_inverse=True)
        S0 = uniq.shape[0]
        norm2 = np.zeros((S0, 2), dtype=np.float32)
        np.add.at(norm2, (inv, (r & 1).astype(np.int64)), nm)
        slot_dl = (uniq // npair).astype(np.int64)
        slot_pr = (uniq % npair).astype(np.int64)
        sdeg = np.bincount(slot_dl, minlength=NDP)
        start_of = np.zeros(NDP + 1, dtype=np.int64)
        np.cumsum(sdeg, out=start_of[1:])
        j_rank = np.arange(S0, dtype=np.int64) - start_of[slot_dl]
        perm = np.argsort(-sdeg, kind="stable")       # position -> dst
        pos_of = np.empty(NDP, dtype=np.int64)
        pos_of[perm] = np.arange(NDP)
        sdeg_tiles[c] = sdeg[perm].reshape(NT, P).max(axis=1)
        per_core.append(dict(slot_dl=slot_dl, slot_pr=slot_pr, j_rank=j_rank,
                             norm2=norm2, pos_of=pos_of, perm=perm))

    cols_t = sdeg_tiles.max(axis=0).astype(np.int64)  # ragged, may be 0
    colbase = np.zeros(NT + 1, dtype=np.int64)
    np.cumsum(cols_t, out=colbase[1:])
    C = int(colbase[-1])

    # matmul pieces: <=2 pair-columns, 2-aligned to their tile's first column
    pieces = []  # (tile, col_lo_global, ncols, first_of_tile, last_of_tile)
    for t in range(NT):
        left = int(cols_t[t])
        c0 = int(colbase[t])
        while left > 0:
            w = min(2, left)
            pieces.append([t, c0, w, c0 == int(colbase[t]),
                           left - w == 0])
            c0 += w
            left -= w
    # pack consecutive pieces into gather calls of <= CALL_COLS columns
    calls = []   # (col_lo_global, ncols, [piece indices])
    cur = None
    for pi, (t, c0, w, fo, lo) in enumerate(pieces):
        if cur is None or cur[1] + w > CALL_COLS:
            cur = [c0, 0, []]
            calls.append(cur)
        cur[1] += w
        cur[2].append(pi)

    in_maps = []
    unshard = []
    for c in range(NCORES):
        pc = per_core[c]
        pos = pc["pos_of"][pc["slot_dl"]]
        prow = pos % P
        scol = colbase[pos // P] + pc["j_rank"]
        idx_arr = np.full((P, max(C, 1)), zero_pair, dtype=np.int16)
        idx_arr[prow, scol] = pc["slot_pr"].astype(np.int16)
        norm2_arr = np.zeros((P, 2 * max(C, 1)), dtype=BF)
        norm2_arr[prow, 2 * scol] = pc["norm2"][:, 0].astype(BF)
        norm2_arr[prow, 2 * scol + 1] = pc["norm2"][:, 1].astype(BF)
        # idx re-layout: per call, slot i (= colj*128 + p over the call's
        # columns) lives at [i%16, base*8 + i//16], replicated over 8 row-groups
        blocks = []
        for (c0, w, _ps) in calls:
            blk = idx_arr[:, c0:c0 + w]                       # [128, w]
            v = blk.T.reshape(-1)                             # slot-major
            b = v.reshape(w * 8, 16).T                        # [16, w*8]
            blocks.append(np.tile(b, (8, 1)))
        idx16 = np.concatenate(blocks, axis=1) if blocks else np.zeros((P, 8), np.int16)
        # self-loop inputs: x rows in position order + dinv^2 per position
        nd_c = min(ND, N - c * ND)
        xs = np.zeros((NDP, F), dtype=BF)
        d2 = np.zeros(NDP, dtype=np.float32)
        valid = pc["perm"] < nd_c
        gids = c * ND + pc["perm"][valid]
        xs[valid] = x[gids].astype(BF)
        d2[valid] = dinv[gids] ** 2
        xself = np.ascontiguousarray(
            xs.reshape(NT, P, F).transpose(1, 0, 2).reshape(P, NT * F))
        dinv2 = np.ascontiguousarray(
            d2.reshape(NT, P).T.astype(BF))                   # [128, NT]
        in_maps.append({
            "ypair": ypair,
            "idx": np.ascontiguousarray(idx16),
            "norm2": np.ascontiguousarray(norm2_arr),
            "xself": xself,
            "dinv2": dinv2,
            "ident": np.eye(P, dtype=BF),
            "w0": W0.astype(BF),
            "w1lo": W1[:128].astype(BF),
            "w1hi": W1[128:].astype(BF),
        })
        unshard.append(pc["pos_of"])

    meta = dict(N=N, F=F, H=H, ND=ND, NT=NT, NDP=NDP, npair=npair,
                C=max(C, 1), cols_t=cols_t.tolist(), colbase=colbase.tolist(),
                calls=calls, pieces=pieces, idx_cols=sum(w * 8 for (_c, w, _ps) in calls))
    return in_maps, unshard, meta


def _build(meta):
    F, H = meta["F"], meta["H"]
    NT, npair = meta["NT"], meta["npair"]
    C, cols_t, colbase = meta["C"], meta["cols_t"], meta["colbase"]
    calls = meta["calls"]
    idx_cols = meta["idx_cols"]
    F2 = 2 * F

    nc = bacc.Bacc(None, target_bir_lowering=False, debug=False,
                   num_devices=NCORES, num_swdge_queues=NQ,
                   dynamic_dma_scratch_size=NQ * CALL_COLS * P * 16)
    ypair_d = nc.declare_dram_parameter("ypair", [npair, F2], bf16, isOutput=False)
    idx_d = nc.declare_dram_parameter("idx", [P, idx_cols], i16, isOutput=False)
    norm2_d = nc.declare_dram_parameter("norm2", [P, 2 * C], bf16, isOutput=False)
    xself_d = nc.declare_dram_parameter("xself", [P, NT * F], bf16, isOutput=False)
    dinv2_d = nc.declare_dram_parameter("dinv2", [P, NT], bf16, isOutput=False)
    ident_d = nc.declare_dram_parameter("ident", [P, P], bf16, isOutput=False)
    w0_d = nc.declare_dram_parameter("w0", [F, H], bf16, isOutput=False)
    w1lo_d = nc.declare_dram_parameter("w1lo", [128, H], bf16, isOutput=False)
    w1hi_d = nc.declare_dram_parameter("w1hi", [H - 128, H], bf16, isOutput=False)
    out_d = nc.declare_dram_parameter("out", [H, NT * P], f32, isOutput=True)

    chunks = [(j * 4, min(4, NT - j * 4)) for j in range((NT + 3) // 4)]

    with tile.TileContext(nc) as tc, ExitStack() as ctx:
        cpool = ctx.enter_context(tc.tile_pool(name="const", bufs=1))
        gpool = ctx.enter_context(tc.tile_pool(name="g", bufs=3))
        spool = ctx.enter_context(tc.tile_pool(name="gs", bufs=3))
        hpool = ctx.enter_context(tc.tile_pool(name="h0", bufs=2))
        h0Tp = ctx.enter_context(tc.tile_pool(name="h0T", bufs=3))
        h1p = ctx.enter_context(tc.tile_pool(name="h1", bufs=2))
        opool = ctx.enter_context(tc.tile_pool(name="o", bufs=2))
        ps_acc = ctx.enter_context(tc.tile_pool(name="ps_acc", bufs=2, space="PSUM"))
        ps_tr = ctx.enter_context(tc.tile_pool(name="ps_tr", bufs=2, space="PSUM"))
        ps_u = ctx.enter_context(tc.tile_pool(name="ps_u", bufs=1, space="PSUM"))
        ps_v = ctx.enter_context(tc.tile_pool(name="ps_v", bufs=1, space="PSUM"))

        ident = cpool.tile([P, P], bf16)
        nc.sync.dma_start(ident[:], ident_d[:])
        # split the index/norm prologue loads so the first gathers start early
        idx_sb = cpool.tile([P, idx_cols], i16)
        n_head = min(idx_cols, 16 * 8)
        nc.sync.dma_start(idx_sb[:, :n_head], idx_d[:, :n_head])
        if idx_cols > n_head:
            nc.sync.dma_start(idx_sb[:, n_head:], idx_d[:, n_head:])
        norm2_sb = cpool.tile([P, 2 * C], bf16)
        c_head = min(2 * C, 2 * 64)
        nc.sync.dma_start(norm2_sb[:, :c_head], norm2_d[:, :c_head])
        if 2 * C > c_head:
            nc.sync.dma_start(norm2_sb[:, c_head:], norm2_d[:, c_head:])
        xself_sb = cpool.tile([P, NT * F], bf16)
        nc.sync.dma_start(xself_sb[:], xself_d[:])
        dinv2_sb = cpool.tile([P, NT], bf16)
        nc.sync.dma_start(dinv2_sb[:], dinv2_d[:])
        w0_sb = cpool.tile([F, H], bf16)
        nc.sync.dma_start(w0_sb[:], w0_d[:])
        w1lo_sb = cpool.tile([128, H], bf16)
        nc.sync.dma_start(w1lo_sb[:], w1lo_d[:])
        w1hi_sb = cpool.tile([H - 128, H], bf16)
        nc.sync.dma_start(w1hi_sb[:], w1hi_d[:])

        # self-loop columns: xself * dinv2 (broadcast along features), in place
        selfcols = xself_sb
        nc.vector.tensor_tensor(out=selfcols[:], in0=xself_sb[:],
                                in1=dinv2_sb[:].to_broadcast([P, NT, F]),
                                op=mybir.AluOpType.mult)

        h0T_chunk = {}

        def finish_tile(t, accp, nquad):
            h0tmp = hpool.tile([P, P], bf16, tag="h0tmp")
            in_ap = AP(accp[:].tensor, accp[:].offset,
                       [accp[:].ap[0], [1, P], [P, nquad]])
            with nc.allow_low_precision("bf16 h0 evict"):
                nc.vector.tensor_reduce(h0tmp[:], in_ap, axis=mybir.AxisListType.X,
                                        op=mybir.AluOpType.add, opt_input=False)
            trp = ps_tr.tile([P, P], bf16, tag="tr")
            nc.tensor.transpose(trp[:], h0tmp[:], ident[:])
            j = t // 4
            if j not in h0T_chunk:
                w = chunks[j][1] * P
                h0T_new = h0Tp.tile([P, w], bf16, tag="h0T")
                h0T_chunk[j] = h0T_new
            nc.scalar.copy(h0T_chunk[j][:, (t % 4) * P:(t % 4 + 1) * P], trp[:])
            if t % 4 == 3 or t == NT - 1:
                phase2(j)

        def phase2(j):
            t0, ntile = chunks[j]
            w = ntile * P
            h0T = h0T_chunk.pop(j)
            u1 = ps_u.tile([P, w], f32, tag="u1")
            u2 = ps_u.tile([P, w], f32, tag="u2")
            nc.tensor.matmul(u1[:], lhsT=w0_sb[:, 0:128], rhs=h0T[:], start=True, stop=True)
            nc.tensor.matmul(u2[:], lhsT=w0_sb[:, 128:H], rhs=h0T[:], start=True, stop=True)
            h1a = h1p.tile([P, w], bf16, tag="h1a")
            h1b = h1p.tile([P, w], bf16, tag="h1b")
            nc.scalar.activation(h1a[:], u1[:], mybir.ActivationFunctionType.Relu)
            nc.scalar.activation(h1b[:], u2[:], mybir.ActivationFunctionType.Relu)
            v1 = ps_v.tile([P, w], f32, tag="v1")
            v2 = ps_v.tile([P, w], f32, tag="v2")
            nc.tensor.matmul(v1[:], lhsT=w1lo_sb[:, 0:128], rhs=h1a[:], start=True, stop=False)
            nc.tensor.matmul(v1[:], lhsT=w1hi_sb[:, 0:128], rhs=h1b[:], start=False, stop=True)
            nc.tensor.matmul(v2[:], lhsT=w1lo_sb[:, 128:H], rhs=h1a[:], start=True, stop=False)
            nc.tensor.matmul(v2[:], lhsT=w1hi_sb[:, 128:H], rhs=h1b[:], start=False, stop=True)
            o1 = opool.tile([P, w], f32, tag="o1")
            o2 = opool.tile([P, w], f32, tag="o2")
            nc.scalar.activation(o1[:], v1[:], mybir.ActivationFunctionType.Relu)
            nc.scalar.activation(o2[:], v2[:], mybir.ActivationFunctionType.Relu)
            nc.sync.dma_start(out_d[0:128, t0 * P:t0 * P + w], o1[:])
            nc.sync.dma_start(out_d[128:H, t0 * P:t0 * P + w], o2[:])

        # phase 1: piece-packed gather calls
        pieces = meta["pieces"]

        def self_only_tile(t):
            acc0 = ps_acc.tile([P, 4 * F], f32, tag="acc")
            nc.tensor.matmul(acc0[:, 0:F], lhsT=ident[:],
                             rhs=selfcols[:, t * F:(t + 1) * F],
                             start=True, stop=True)
            finish_tile(t, acc0, 1)

        next_tile = 0            # next tile expected to start
        acc_of = {}              # tile -> psum acc
        idx_base = 0
        kq = 0
        for (cstart, cw, plist) in calls:
            g = gpool.tile([P, CALL_COLS * F2], bf16, tag="g")
            nidx = cw * P
            nc.gpsimd.dma_gather(
                out_ap=_ap3(g[:, :cw * F2], cw, F2),
                in_ap=ypair_d[:],
                idxs_ap=idx_sb[:, idx_base:idx_base + cw * 8],
                num_idxs=nidx, num_idxs_reg=nidx, elem_size=F2,
                single_packet=False, queue_num=kq % NQ)
            kq += 1
            idx_base += cw * 8
            gs = spool.tile([P, CALL_COLS * F2], bf16, tag="gs")
            nc.vector.tensor_tensor(
                out=gs[:, :cw * F2], in0=g[:, :cw * F2],
                in1=norm2_sb[:, 2 * cstart:2 * (cstart + cw)]
                    .to_broadcast([P, 2 * cw, F]),
                op=mybir.AluOpType.mult)
            for pi in plist:
                t, c0, pw, first_of_t, last_of_t = pieces[pi]
                if first_of_t:
                    while next_tile < t:     # tiles with no gather columns
                        self_only_tile(next_tile)
                        next_tile += 1
                    accnew = ps_acc.tile([P, 4 * F], f32, tag="acc")
                    acc_of[t] = accnew
                    next_tile = t + 1
                accp = acc_of[t]
                off = c0 - cstart
                nc.tensor.matmul(accp[:, :pw * F2], lhsT=ident[:],
                                 rhs=gs[:, off * F2:(off + pw) * F2],
                                 start=first_of_t,
                                 stop=(last_of_t and not first_of_t))
                if first_of_t:
                    # self-loop column rides in quarter 0
                    nc.tensor.matmul(accp[:, 0:F], lhsT=ident[:],
                                     rhs=selfcols[:, t * F:(t + 1) * F],
                                     start=False, stop=last_of_t)
                if last_of_t:
                    finish_tile(t, acc_of.pop(t), min(4, 2 * cols_t[t]))
        while next_tile < NT:
            self_only_tile(next_tile)
            next_tile += 1
    nc.compile()
    return nc


def _run(inputs, trace=False):
    x = np.asarray(inputs["x"])
    W0 = np.asarray(inputs["W0"])
    W1 = np.asarray(inputs["W1"])
    edge_index = np.asarray(inputs["edge_index"])
    in_maps, unshard, meta = _prep(x, W0, W1, edge_index)
    nc = _build(meta)
    res = run_bass_kernel_spmd(nc, in_maps, core_ids=list(range(NCORES)), trace=trace)
    N, H, ND = meta["N"], meta["H"], meta["ND"]
    h = np.empty((N, H), dtype=np.float32)
    for c in range(NCORES):
        o = res.results[c]["out"]            # [H, NT*P]
        nd_c = min(ND, N - c * ND)
        h[c * ND:c * ND + nd_c] = o.T[unshard[c][:nd_c]]
    return h, res


def kernel(**inputs) -> np.ndarray:
    h, _ = _run(inputs, trace=False)
    return h
